# revision 6
# baseline (speedup 1.0000x reference)
"""Trainium2 Bass kernel for a pre-LN transformer decode layer.

nn_DecodeLayer: x [4, 2048, 1024] f32, 16 heads, causal attention, 4x MLP.

Sharding: 8 cores = 4 batch x 2 query-shards. Core c handles batch c%4 and
query tiles {2j + c//4 : j in 0..7} (interleaved 128-row tiles, balancing
causal attention work across the two shards of a batch).

Each core's x copy is column-PERMUTED on the host: its own 1024 query
columns first (in j order), partner's 1024 after. This makes the program
parity-uniform: Q reads the full-seq layernorm output directly (no second
chunk layernorm), key tile t<8 is "own" (causal: visible to q-slot j>=t,
triangle-masked at j==t) and t>=8 is "partner" (visible to j>t-8; the
boundary slot j==t-8 is scaled by a per-core all-ones/all-zeros mask).

On-chip layout: activations transposed ([e, seq]); scoresT = K @ Q^T in
[key, query] layout so probs feed attn@V with no transposes; softmax
denominators via a ones-column in V; head pairs interleave so their
64-contraction QK matmuls land on PE row-groups (0,*) and (64,*) and run
concurrently; psO is a 4-ring so normalization never stalls the PE.
LN gains and 1/sqrt(d) are folded into weights/biases on the host.
"""

import sys

for _p in ("/opt/trn_rl_repo",):
    if _p not in sys.path:
        sys.path.insert(0, _p)

import numpy as np
import ml_dtypes

import concourse.bass as bass
import concourse.tile as tile
from concourse import bacc, mybir
from concourse.bass_utils import run_bass_kernel_spmd

F32 = mybir.dt.float32
BF16 = mybir.dt.bfloat16

E = 1024          # d_model
S = 2048          # sequence length
BATCH = 4
NH = 16           # heads
HD = 64           # head dim
P = 128
ET = E // P       # 8 e-tiles
QC = 1024         # queries per core
NKT = S // P      # 16 key tiles
FF = 4 * E        # 4096
HT = FF // P      # 32 hidden tiles
N_CORES = 8
EPS = 1e-5


def _attn_segs(hf):
    """(tile, col_start, len, mask) for query-half hf; mask in {None,'tri','par'}."""
    out = []
    for t in range(NKT):
        jm = t if t < 8 else t - 8
        cs = max(jm * P, hf * 512)
        ce = (hf + 1) * 512
        if cs >= ce:
            continue
        mask = None
        if cs == jm * P:
            mask = "tri" if t < 8 else "par"
        out.append((t, cs, ce - cs, mask))
    return out


def build_program(repeat=1):
    nc = bacc.Bacc("TRN2", num_devices=N_CORES)

    d = {}
    def din(name, shape, dtype):
        d[name] = nc.dram_tensor(name, shape, dtype, kind="ExternalInput").ap()

    din("x_full_bf", [E, S], BF16)     # permuted x[b].T (own cols first), bf16
    din("x_chunk", [E, QC], F32)       # own query cols (f32 residual)
    din("wq", [E, E], BF16)            # ln1_g-folded, /8-folded
    din("wk", [E, E], BF16)
    din("wv", [E, E], BF16)
    din("wproj", [E, E], BF16)
    din("wfc", [E, FF], BF16)          # ln2_g-folded
    din("wfc2", [FF, E], BF16)
    din("bq", [E], F32)
    din("bk", [E], F32)
    din("bv", [E], F32)
    din("bproj", [E], F32)
    din("bfc", [FF], F32)
    din("bfc2", [E], F32)
    din("tri_mask", [P, P], BF16)      # causal triangle (q >= k)
    din("par_mask", [P, P], BF16)      # all-ones (parity 1) / all-zeros (parity 0)
    out_ap = nc.dram_tensor("out", [E, QC], F32, kind="ExternalOutput").ap()

    with tile.TileContext(nc) as tc:
        if repeat == 1:
            _emit(nc, tc, d, out_ap)
        else:
            with tc.For_i(0, repeat, 1):
                _emit(nc, tc, d, out_ap)

    nc.compile()
    return nc


def _emit(nc, tc, d, out_ap):
    A = mybir.ActivationFunctionType
    O = mybir.AluOpType
    import contextlib
    ctx = contextlib.ExitStack()
    with ctx:
        # --- long-lived pools ---
        pconst = ctx.enter_context(tc.tile_pool(name="pconst", bufs=1))
        pbig = ctx.enter_context(tc.tile_pool(name="pbig", bufs=1))
        pxb = ctx.enter_context(tc.tile_pool(name="pxb", bufs=3))
        prows = ctx.enter_context(tc.tile_pool(name="prows", bufs=2))
        postg = ctx.enter_context(tc.tile_pool(name="postg", bufs=3))

        # --- constants ---
        ones_mat = pconst.tile([P, P], BF16, tag="ones")
        nc.vector.memset(ones_mat, 1.0)
        eps_t = pconst.tile([P, 1], F32, tag="eps")
        nc.vector.memset(eps_t, EPS)
        tri_sb = pconst.tile([P, P], BF16, tag="tri")
        nc.sync.dma_start(out=tri_sb, in_=d["tri_mask"])
        par_sb = pconst.tile([P, P], BF16, tag="par")
        nc.sync.dma_start(out=par_sb, in_=d["par_mask"])

        def bias_cols(name, n_tiles):
            t = pconst.tile([P, n_tiles], F32, tag=f"b_{name}", name=f"b_{name}")
            nc.sync.dma_start(out=t, in_=d[name].rearrange("(t p) -> p t", p=P))
            return t

        bq_sb = bias_cols("bq", ET)
        bk_sb = bias_cols("bk", ET)
        bproj_sb = bias_cols("bproj", ET)
        bfc2_sb = bias_cols("bfc2", ET)
        bfc_sb = bias_cols("bfc", HT)

        # bv as a broadcast row [P, E] (bias varies along free = v-channel)
        bvrow = prows.tile([1, E], F32, tag="rows", name="rows")
        nc.sync.dma_start(out=bvrow, in_=d["bv"].rearrange("(o n) -> o n", o=1))
        bvrow_bf = prows.tile([1, E], BF16, tag="rows_bf", name="rows_bf")
        nc.gpsimd.tensor_copy(bvrow_bf, bvrow)
        bvb = pconst.tile([P, E], BF16, tag="bvb")
        nc.gpsimd.partition_broadcast(bvb, bvrow_bf)

        # --- big SBUF tiles ---
        xnf_h = [pbig.tile([P, ET, 1024], BF16, tag=f"T1{i}", name=f"T1{i}")
                 for i in range(2)]
        KT = pbig.tile([P, ET, S], BF16, tag="T3")
        QT = pbig.tile([P, ET, QC], BF16, tag="T4")
        VA = pbig.tile([P, NKT, NH * (HD + 1)], BF16, tag="T5")
        attnT = pbig.tile([P, ET, QC], BF16, tag="T2")

        # ones column of VA (softmax denominator rows), one strided memset
        nc.gpsimd.memset(
            VA.rearrange("p t (h c) -> p t h c", c=HD + 1)[:, :, :, HD:HD + 1],
            1.0)

        # --- layernorm for one 512-col block (stats via all-ones stationary;
        # everything 128-wide, means/rstds partition-broadcast by the matmul) ---
        def ln_block(pst, pbc, dst, dcol0, w, src_dram=None, src_tiles=None,
                     xh_pool=None):
            if src_dram is not None:
                xh = xh_pool.tile([P, ET, 512], BF16, tag="xh", name="xh")[:, :, :w]
                nc.sync.dma_start(out=xh,
                                  in_=src_dram.rearrange("(t p) c -> p t c", p=P))
            ps_x = pst.tile([P, 512], F32, tag="st_x", name="st_x")[:, :w]
            ps_q = pst.tile([P, 512], F32, tag="st_q", name="st_q")[:, :w]
            for et in range(ET):
                if src_dram is not None:
                    xt = xh[:, et, :]
                else:
                    xt = pxb.tile([P, 512], BF16, tag="xb", name="xb")[:, :w]
                    nc.gpsimd.tensor_copy(xt, src_tiles(et))
                sq = pxb.tile([P, 512], BF16, tag="xb", name="xb")[:, :w]
                nc.scalar.activation(sq, xt, A.Square)
                nc.tensor.matmul(ps_x, ones_mat, xt,
                                 start=(et == 0), stop=(et == ET - 1))
                nc.tensor.matmul(ps_q, ones_mat, sq,
                                 start=(et == 0), stop=(et == ET - 1))
            m_t = pbc.tile([P, 512], BF16, tag="bc", name="bc")[:, :w]
            nc.scalar.activation(m_t, ps_x, A.Copy, scale=1.0 / E)
            e2_t = pbc.tile([P, 512], F32, tag="bcf", name="bcf")[:, :w]
            nc.scalar.activation(e2_t, ps_q, A.Copy, scale=1.0 / E)
            var_t = pbc.tile([P, 512], F32, tag="bcf", name="bcf")[:, :w]
            nc.vector.scalar_tensor_tensor(var_t, in0=m_t, scalar=-1.0,
                                           in1=m_t, op0=O.mult, op1=O.mult)
            nc.vector.tensor_add(var_t, var_t, e2_t)
            nc.scalar.activation(var_t, var_t, A.Sqrt, bias=eps_t)
            r_t = pbc.tile([P, 512], F32, tag="bcf", name="bcf")[:, :w]
            nc.vector.reciprocal(r_t, var_t)
            for et in range(ET):
                xt = xh[:, et, :] if src_dram is not None else src_tiles(et)
                dst_v = dst[:, et, dcol0:dcol0 + w]
                eng = nc.gpsimd if et % 2 == 1 else nc.vector
                eng.tensor_sub(dst_v, xt, m_t)
                eng.tensor_mul(dst_v, dst_v, r_t)

        # ---- phase 1: LN1 over the permuted full seq, V interleaved ----
        with tc.tile_pool(name="pst1", bufs=2, space="PSUM") as pst, \
             tc.tile_pool(name="pvmm", bufs=2, space="PSUM") as pvm, \
             tc.tile_pool(name="pbc1", bufs=4) as pbc, \
             tc.tile_pool(name="pxh1", bufs=2) as pxh, \
             tc.tile_pool(name="pwv", bufs=1) as pwv:
            wv_sb = [pwv.tile([P, ET, 512], BF16, tag=f"wv{vh}", name=f"wv{vh}")
                     for vh in range(2)]
            for vh in range(2):
                nc.sync.dma_start(
                    out=wv_sb[vh],
                    in_=d["wv"][:, vh * 512:(vh + 1) * 512]
                    .rearrange("(t p) c -> p t c", p=P))
            for blk in range(4):
                c0 = blk * 512
                ln_block(pst, pbc, xnf_h[blk // 2], (c0 % 1024), 512,
                         src_dram=d["x_full_bf"][:, c0:c0 + 512], xh_pool=pxh)
                # V for the 4 key tiles of this block
                for t in range(4 * blk, 4 * blk + 4):
                    xn_src = xnf_h[t // 8]
                    for vh in range(2):
                        hbase = vh * (NH // 2)
                        ps = pvm.tile([P, 512], F32, tag="vmm", name="vmm")
                        for et in range(ET):
                            nc.tensor.matmul(
                                ps, xn_src[:, et, (t % 8) * P:(t % 8 + 1) * P],
                                wv_sb[vh][:, et, :],
                                start=(et == 0), stop=(et == ET - 1))
                        va_v = VA[:, t, hbase * (HD + 1):(hbase + 8) * (HD + 1)] \
                            .rearrange("p (h c) -> p h c", c=HD + 1)
                        nc.vector.tensor_add(
                            va_v[:, :, 0:HD],
                            ps.rearrange("p (h c) -> p h c", c=HD),
                            bvb[:, vh * 512:(vh + 1) * 512]
                            .rearrange("p (h c) -> p h c", c=HD))

        # ---- phase 2: per kd: K, Q, then the attention head pair ----
        with tc.tile_pool(name="pwc", bufs=3) as pw, \
             tc.tile_pool(name="pprobs", bufs=6) as pprobs, \
             tc.tile_pool(name="prb", bufs=4) as prb, \
             tc.tile_pool(name="psc", bufs=4, space="PSUM") as psc, \
             tc.tile_pool(name="pso", bufs=4, space="PSUM") as pso:
            for kd in range(ET):
                # K then Q projections for this kd column
                for (wname, bcol, dst, scols) in (
                        ("wk", bk_sb, KT, S), ("wq", bq_sb, QT, QC)):
                    wt = pw.tile([P, ET, P], BF16, tag="wcol", name="wcol")
                    nc.sync.dma_start(
                        out=wt,
                        in_=d[wname][:, kd * P:(kd + 1) * P]
                        .rearrange("(t p) c -> p t c", p=P))
                    for c0 in range(0, scols, 512):
                        ps = psc.tile([P, 512], F32, tag="sc", name="sc")
                        for et in range(ET):
                            s_ap = xnf_h[c0 // 1024][:, et,
                                                     c0 % 1024:c0 % 1024 + 512]
                            nc.tensor.matmul(ps, wt[:, et, :], s_ap,
                                             start=(et == 0), stop=(et == ET - 1))
                        nc.vector.tensor_scalar(
                            dst[:, kd, c0:c0 + 512], ps,
                            bcol[:, kd:kd + 1], None, op0=O.add)

                # attention for heads (2kd, 2kd+1), interleaved per seg
                h0, h1 = 2 * kd, 2 * kd + 1
                for hf in range(2):
                    segs = _attn_segs(hf)
                    last_t = segs[-1][0]
                    psO = [pso.tile([HD + 1, 512], F32, tag="psO",
                                    name=f"psO{hh}") for hh in range(2)]
                    for (t, cs, ln, mask) in segs:
                        probs = []
                        for hh, off in ((0, 0), (1, HD)):
                            sc = psc.tile([P, 512], F32, tag="sc",
                                          name="sc")[:, :ln]
                            nc.tensor.matmul(
                                sc,
                                KT[off:off + HD, kd, t * P:(t + 1) * P],
                                QT[off:off + HD, kd, cs:cs + ln],
                                start=True, stop=True)
                            pr = pprobs.tile([P, 512], BF16, tag="probs",
                                             name="probs")[:, :ln]
                            nc.scalar.activation(pr, sc, A.Exp)
                            if mask == "tri":
                                eng = nc.vector if hh == 0 else nc.gpsimd
                                eng.tensor_mul(pr[:, 0:P], pr[:, 0:P], tri_sb)
                            elif mask == "par":
                                eng = nc.vector if hh == 0 else nc.gpsimd
                                eng.tensor_mul(pr[:, 0:P], pr[:, 0:P], par_sb)
                            probs.append(pr)
                        for hh, h in ((0, h0), (1, h1)):
                            nc.tensor.matmul(
                                psO[hh][:, cs - hf * 512:512],
                                VA[:, t, h * (HD + 1):(h + 1) * (HD + 1)],
                                probs[hh],
                                start=(t == 0), stop=(t == last_t),
                                skip_group_check=True)
                    # normalization (never blocks the PE: psO is a 4-ring)
                    for hh in range(2):
                        off = hh * HD
                        rrow = prows.tile([1, 512], BF16, tag="rows_bf",
                                          name="rrow")
                        with nc.allow_low_precision(
                                reason="recip row feeds bf16 mul; same "
                                       "precision as f32-recip-then-cast"):
                            nc.vector.reciprocal(rrow, psO[hh][HD:HD + 1, :])
                        rb = prb.tile([HD, 512], BF16, tag="rb", name="rb")
                        nc.gpsimd.partition_broadcast(rb, rrow)
                        nc.vector.tensor_mul(
                            attnT[off:off + HD, kd, hf * 512:(hf + 1) * 512],
                            psO[hh][0:HD, :], rb)

        # ---- phase 3: proj + residual -> x2, LN2 -> xn2 (per q-half) ----
        x2_h = [pbig.tile([P, ET, 512], F32, tag=f"T1{i}", name=f"x2{i}")
                for i in range(2)]
        xn2 = pbig.tile([P, ET, QC], BF16, tag="T4")
        with tc.tile_pool(name="pwp", bufs=3) as pw, \
             tc.tile_pool(name="pppr", bufs=2, space="PSUM") as ppp, \
             tc.tile_pool(name="pst2", bufs=2, space="PSUM") as pst, \
             tc.tile_pool(name="pbc2", bufs=4) as pbc:
            for qh in range(2):
                c0 = qh * 512
                for et in range(ET):
                    wt = pw.tile([P, ET, P], BF16, tag="wcol", name="wcol")
                    nc.sync.dma_start(
                        out=wt,
                        in_=d["wproj"][:, et * P:(et + 1) * P]
                        .rearrange("(t p) c -> p t c", p=P))
                    ps = ppp.tile([P, 512], F32, tag="mm", name="mm")
                    for hd in range(ET):
                        nc.tensor.matmul(ps, wt[:, hd, :],
                                         attnT[:, hd, c0:c0 + 512],
                                         start=(hd == 0), stop=(hd == ET - 1))
                    xc = postg.tile([P, 512], F32, tag="ostg", name="ostg")
                    nc.sync.dma_start(
                        out=xc,
                        in_=d["x_chunk"][et * P:(et + 1) * P, c0:c0 + 512])
                    nc.vector.scalar_tensor_tensor(
                        x2_h[qh][:, et, :], in0=ps,
                        scalar=bproj_sb[:, et:et + 1], in1=xc,
                        op0=O.add, op1=O.add)
                ln_block(pst, pbc, xn2, c0, 512,
                         src_tiles=lambda et: x2_h[qh][:, et, :])

        # ---- phase 4: FFN, weight-single-pass (qh inner) ----
        Hsb = [pbig.tile([P, HT, 512], BF16, tag="T3", name="HsbA"),
               pbig.tile([P, HT, 512], BF16, tag="T5", name="HsbB")]
        with tc.tile_pool(name="pwf", bufs=2) as pwf, \
             tc.tile_pool(name="pwf2", bufs=4) as pwf2, \
             tc.tile_pool(name="ppf1", bufs=2, space="PSUM") as ppf1, \
             tc.tile_pool(name="ppf2", bufs=4, space="PSUM") as ppf2:
            for hg in range(8):
                wt = pwf.tile([P, ET, 512], BF16, tag="wfc1", name="wfc1")
                nc.sync.dma_start(
                    out=wt,
                    in_=d["wfc"][:, hg * 512:(hg + 1) * 512]
                    .rearrange("(t p) c -> p t c", p=P))
                for qh in range(2):
                    for h4 in range(4):
                        ht = hg * 4 + h4
                        ps = ppf1.tile([P, 512], F32, tag="mmh", name="mmh")
                        for et in range(ET):
                            nc.tensor.matmul(
                                ps, wt[:, et, h4 * P:(h4 + 1) * P],
                                xn2[:, et, qh * 512:qh * 512 + 512],
                                start=(et == 0), stop=(et == ET - 1))
                        nc.scalar.activation(Hsb[qh][:, ht, :], ps, A.Gelu,
                                             bias=bfc_sb[:, ht:ht + 1])
            for qh in range(2):
                for eg in range(2):
                    psY = [ppf2.tile([P, 512], F32, tag="psY",
                                     name=f"psY{i}") for i in range(4)]
                    for ht in range(HT):
                        wt = pwf2.tile([P, 512], BF16, tag="wfc2", name="wfc2")
                        nc.sync.dma_start(
                            out=wt,
                            in_=d["wfc2"][ht * P:(ht + 1) * P,
                                          eg * 512:(eg + 1) * 512])
                        for e4 in range(4):
                            nc.tensor.matmul(
                                psY[e4], wt[:, e4 * P:(e4 + 1) * P],
                                Hsb[qh][:, ht, :],
                                start=(ht == 0), stop=(ht == HT - 1))
                    for e4 in range(4):
                        et = eg * 4 + e4
                        og = postg.tile([P, 512], F32, tag="ostg", name="ostg")
                        nc.vector.scalar_tensor_tensor(
                            og, in0=psY[e4], scalar=bfc2_sb[:, et:et + 1],
                            in1=x2_h[qh][:, et, :],
                            op0=O.add, op1=O.add)
                        nc.sync.dma_start(
                            out=out_ap[et * P:(et + 1) * P,
                                       qh * 512:qh * 512 + 512],
                            in_=og)

# ---------------------------------------------------------------------------
# host side
# ---------------------------------------------------------------------------

_PROG_CACHE = {}


def get_program(repeat=1):
    key = repeat
    if key not in _PROG_CACHE:
        _PROG_CACHE[key] = build_program(repeat)
    return _PROG_CACHE[key]


def _own_rows(parity):
    return np.concatenate(
        [np.arange(P * (2 * j + parity), P * (2 * j + parity) + P)
         for j in range(8)])


def prep_in_maps(x, ln1_g, ln1_b, w_attn, b_attn, w_proj, b_proj,
                 ln2_g, ln2_b, w_fc, b_fc, w_fc2, b_fc2):
    f32 = np.float32
    bf = ml_dtypes.bfloat16
    x = np.asarray(x, f32)
    g1 = np.asarray(ln1_g, f32)[:, None]
    wq = (g1 * np.asarray(w_attn[:, 0:E], f32)) / 8.0
    wk = g1 * np.asarray(w_attn[:, E:2 * E], f32)
    wv = g1 * np.asarray(w_attn[:, 2 * E:3 * E], f32)
    bq = (np.asarray(w_attn[:, 0:E], f32).T @ np.asarray(ln1_b, f32)
          + np.asarray(b_attn[0:E], f32)) / 8.0
    bk = (np.asarray(w_attn[:, E:2 * E], f32).T @ np.asarray(ln1_b, f32)
          + np.asarray(b_attn[E:2 * E], f32))
    bv = (np.asarray(w_attn[:, 2 * E:3 * E], f32).T @ np.asarray(ln1_b, f32)
          + np.asarray(b_attn[2 * E:3 * E], f32))
    g2 = np.asarray(ln2_g, f32)[:, None]
    wfc = g2 * np.asarray(w_fc, f32)
    bfc = np.asarray(w_fc, f32).T @ np.asarray(ln2_b, f32) + np.asarray(b_fc, f32)

    shared = {
        "wq": np.ascontiguousarray(wq.astype(bf)),
        "wk": np.ascontiguousarray(wk.astype(bf)),
        "wv": np.ascontiguousarray(wv.astype(bf)),
        "wproj": np.ascontiguousarray(np.asarray(w_proj, f32).astype(bf)),
        "wfc": np.ascontiguousarray(wfc.astype(bf)),
        "wfc2": np.ascontiguousarray(np.asarray(w_fc2, f32).astype(bf)),
        "bq": np.ascontiguousarray(bq.astype(f32)),
        "bk": np.ascontiguousarray(bk.astype(f32)),
        "bv": np.ascontiguousarray(bv.astype(f32)),
        "bproj": np.ascontiguousarray(np.asarray(b_proj, f32)),
        "bfc": np.ascontiguousarray(bfc.astype(f32)),
        "bfc2": np.ascontiguousarray(np.asarray(b_fc2, f32)),
    }

    tri = (np.arange(P)[:, None] <= np.arange(P)[None, :]).astype(np.float32)
    tri = np.ascontiguousarray(tri.astype(bf))

    in_maps = []
    for c in range(N_CORES):
        b, parity = c % BATCH, c // BATCH
        rows_own = _own_rows(parity)
        rows_par = _own_rows(1 - parity)
        perm = np.concatenate([rows_own, rows_par])
        xbt = np.ascontiguousarray(x[b].T)          # [E, S]
        m = dict(shared)
        m["x_full_bf"] = np.ascontiguousarray(xbt[:, perm].astype(bf))
        m["x_chunk"] = np.ascontiguousarray(xbt[:, rows_own])
        m["tri_mask"] = tri
        m["par_mask"] = np.ascontiguousarray(
            np.full((P, P), float(parity), np.float32).astype(bf))
        in_maps.append(m)
    return in_maps


def assemble_output(results):
    y = np.empty((BATCH, S, E), np.float32)
    for c in range(N_CORES):
        b, parity = c % BATCH, c // BATCH
        y[b, _own_rows(parity), :] = results[c]["out"].T
    return y


def kernel(**inputs):
    nc = get_program(1)
    in_maps = prep_in_maps(**inputs)
    res = run_bass_kernel_spmd(nc, in_maps, core_ids=list(range(N_CORES)))
    return assemble_output(res.results)


# revision 19
# speedup vs baseline: 1.3560x; 1.3560x over previous
"""Trainium2 Bass kernel for a pre-LN transformer decode layer.

nn_DecodeLayer: x [4, 2048, 1024] f32, 16 heads, causal attention, 4x MLP.

Sharding: 8 cores = 4 batch x 2 query-shards. Core c handles batch c%4 and
query tiles {2j + c//4 : j in 0..7} (interleaved 128-row tiles, balancing
causal attention work across the two shards of a batch).

Each core's x copy is column-PERMUTED on the host: its own 1024 query
columns first (in j order), partner's 1024 after. This makes the program
parity-uniform: Q reads the full-seq layernorm output directly (no second
chunk layernorm), key tile t<8 is "own" (causal: visible to q-slot j>=t,
triangle-masked at j==t) and t>=8 is "partner" (visible to j>t-8; the
boundary slot j==t-8 is scaled by a per-core all-ones/all-zeros mask).

On-chip layout: activations transposed ([e, seq]); scoresT = K @ Q^T in
[key, query] layout so probs feed attn@V with no transposes; softmax
denominators via a ones-column in V; head pairs interleave so their
64-contraction QK matmuls land on PE row-groups (0,*) and (64,*) and run
concurrently; psO is a 4-ring so normalization never stalls the PE.
LN gains and 1/sqrt(d) are folded into weights/biases on the host.
"""

import sys

for _p in ("/opt/trn_rl_repo",):
    if _p not in sys.path:
        sys.path.insert(0, _p)

import numpy as np
import ml_dtypes

import concourse.bass as bass
import concourse.tile as tile
from concourse import bacc, mybir
from concourse.bass_utils import run_bass_kernel_spmd

F32 = mybir.dt.float32
BF16 = mybir.dt.bfloat16
FP8 = mybir.dt.float8e4
WS = 16.0         # fp8 weight pre-scale (folded out via activation scale)

E = 1024          # d_model
S = 2048          # sequence length
BATCH = 4
NH = 16           # heads
HD = 64           # head dim
P = 128
ET = E // P       # 8 e-tiles
QC = 1024         # queries per core
NKT = S // P      # 16 key tiles
FF = 4 * E        # 4096
HT = FF // P      # 32 hidden tiles
N_CORES = 8
EPS = 1e-5


def _attn_segs(hf):
    """(tile, col_start, len, mask) for query-half hf; mask in {None,'tri','par'}."""
    out = []
    for t in range(NKT):
        jm = t if t < 8 else t - 8
        cs = max(jm * P, hf * 512)
        ce = (hf + 1) * 512
        if cs >= ce:
            continue
        mask = None
        if cs == jm * P:
            mask = "tri" if t < 8 else "par"
        out.append((t, cs, ce - cs, mask))
    return out


def build_program(repeat=1):
    nc = bacc.Bacc("TRN2", num_devices=N_CORES)

    d = {}
    def din(name, shape, dtype):
        d[name] = nc.dram_tensor(name, shape, dtype, kind="ExternalInput").ap()

    din("x_full_bf", [E, S], BF16)     # permuted x[b].T (own cols first), bf16
    din("x_chunk", [E, QC], F32)       # own query cols (f32 residual)
    din("wq", [E, E], FP8)             # ln1_g-folded, x16-scaled fp8
    din("wk", [E, E], FP8)
    din("wv", [E, E], FP8)
    din("wproj", [E, E], FP8)
    din("wfc", [E, FF], BF16)          # ln2_g-folded
    din("wfc2", [FF, E], FP8)          # x16-scaled fp8
    din("bq", [E], F32)
    din("bk", [E], F32)
    din("bv", [E], F32)
    din("bproj", [E], F32)
    din("bfc", [FF], F32)
    din("bfc2", [E], F32)
    din("tri_mask", [P, P], BF16)      # causal triangle (q >= k)
    din("par_mask", [P, P], BF16)      # all-ones (parity 1) / all-zeros (parity 0)
    out_ap = nc.dram_tensor("out", [E, QC], F32, kind="ExternalOutput").ap()

    with tile.TileContext(nc) as tc:
        if repeat == 1:
            _emit(nc, tc, d, out_ap)
        else:
            with tc.For_i(0, repeat, 1):
                _emit(nc, tc, d, out_ap)

    nc.compile()
    return nc


def _emit(nc, tc, d, out_ap):
    A = mybir.ActivationFunctionType
    O = mybir.AluOpType
    import contextlib
    ctx = contextlib.ExitStack()
    with ctx:
        # --- long-lived pools ---
        pconst = ctx.enter_context(tc.tile_pool(name="pconst", bufs=1))
        pbig = ctx.enter_context(tc.tile_pool(name="pbig", bufs=1))
        pxb = ctx.enter_context(tc.tile_pool(name="pxb", bufs=3))
        prows = ctx.enter_context(tc.tile_pool(name="prows", bufs=2))
        postg = ctx.enter_context(tc.tile_pool(name="postg", bufs=3))

        # --- constants ---
        ones_mat = pconst.tile([P, P], BF16, tag="ones")
        nc.vector.memset(ones_mat, 1.0)
        eps_t = pconst.tile([P, 1], F32, tag="eps")
        nc.vector.memset(eps_t, EPS)
        tri_sb = pconst.tile([P, P], BF16, tag="tri")
        nc.sync.dma_start(out=tri_sb, in_=d["tri_mask"])
        par_sb = pconst.tile([P, P], BF16, tag="par")
        nc.sync.dma_start(out=par_sb, in_=d["par_mask"])

        def bias_cols(name, n_tiles):
            t = pconst.tile([P, n_tiles], F32, tag=f"b_{name}", name=f"b_{name}")
            nc.sync.dma_start(out=t, in_=d[name].rearrange("(t p) -> p t", p=P))
            return t

        bq_sb = bias_cols("bq", ET)
        bk_sb = bias_cols("bk", ET)
        bproj_sb = bias_cols("bproj", ET)
        bfc2_sb = bias_cols("bfc2", ET)
        bfc_sb = bias_cols("bfc", HT)

        # bv as a broadcast row [P, E] (bias varies along free = v-channel)
        bvrow = prows.tile([1, E], F32, tag="rows", name="rows")
        nc.sync.dma_start(out=bvrow, in_=d["bv"].rearrange("(o n) -> o n", o=1))
        bvrow_bf = prows.tile([1, E], BF16, tag="rows_bf", name="rows_bf")
        nc.gpsimd.tensor_copy(bvrow_bf, bvrow)
        bvb = pconst.tile([P, E], BF16, tag="bvb")
        nc.gpsimd.partition_broadcast(bvb, bvrow_bf)

        # --- big SBUF tiles ---
        xnf_h = [pbig.tile([P, ET, 1024], FP8, tag=f"T1{i}", name=f"T1{i}")
                 for i in range(2)]
        DR = mybir.MatmulPerfMode.DoubleRow
        KT = pbig.tile([P, ET, S], BF16, tag="T3")
        QT = pbig.tile([P, ET, QC], BF16, tag="T4")
        VA = pbig.tile([P, NKT, NH * (HD + 1)], FP8, tag="T5")
        attnT = pbig.tile([P, ET, QC], FP8, tag="T2")

        # ones column of VA (softmax denominator rows), one strided memset
        nc.gpsimd.memset(
            VA.rearrange("p t (h c) -> p t h c", c=HD + 1)[:, :, :, HD:HD + 1],
            WS)

        # --- layernorm for one 512-col block (stats via all-ones stationary;
        # everything 128-wide, means/rstds partition-broadcast by the matmul) ---
        def ln_block(pst, pbc, dst, dcol0, w, src_dram=None, src_tiles=None,
                     xh_pool=None):
            if src_dram is not None:
                xh = xh_pool.tile([P, ET, 512], BF16, tag="xh", name="xh")[:, :, :w]
                nc.sync.dma_start(out=xh,
                                  in_=src_dram.rearrange("(t p) c -> p t c", p=P))
            ps_x = pst.tile([P, 512], F32, tag="st_x", name="st_x")[:, :w]
            ps_q = pst.tile([P, 512], F32, tag="st_q", name="st_q")[:, :w]
            for et in range(ET):
                if src_dram is not None:
                    xt = xh[:, et, :]
                else:
                    xt = pxb.tile([P, 512], BF16, tag="xb", name="xb")[:, :w]
                    nc.gpsimd.tensor_copy(xt, src_tiles(et))
                sq = pxb.tile([P, 512], BF16, tag="xb", name="xb")[:, :w]
                nc.scalar.activation(sq, xt, A.Square)
                nc.tensor.matmul(ps_x, ones_mat, xt,
                                 start=(et == 0), stop=(et == ET - 1))
                nc.tensor.matmul(ps_q, ones_mat, sq,
                                 start=(et == 0), stop=(et == ET - 1))
            m_t = pbc.tile([P, 512], BF16, tag="bc", name="bc")[:, :w]
            nc.scalar.activation(m_t, ps_x, A.Copy, scale=1.0 / E)
            e2_t = pbc.tile([P, 512], F32, tag="bcf", name="bcf")[:, :w]
            nc.scalar.activation(e2_t, ps_q, A.Copy, scale=1.0 / E)
            var_t = pbc.tile([P, 512], F32, tag="bcf", name="bcf")[:, :w]
            nc.vector.scalar_tensor_tensor(var_t, in0=m_t, scalar=-1.0,
                                           in1=m_t, op0=O.mult, op1=O.mult)
            nc.vector.tensor_add(var_t, var_t, e2_t)
            nc.scalar.activation(var_t, var_t, A.Sqrt, bias=eps_t)
            r_t = pbc.tile([P, 512], F32, tag="bcf", name="bcf")[:, :w]
            nc.vector.reciprocal(r_t, var_t)
            for et in range(ET):
                xt = xh[:, et, :] if src_dram is not None else src_tiles(et)
                dst_v = dst[:, et, dcol0:dcol0 + w]
                eng = nc.gpsimd if et % 2 == 1 else nc.vector
                if dst.dtype == FP8:
                    tmp = pxb.tile([P, 512], BF16, tag="xb", name="xb")[:, :w]
                    eng.tensor_sub(tmp, xt, m_t)
                    with nc.allow_low_precision(reason="fp8 xn feed"):
                        eng.tensor_mul(dst_v, tmp, r_t)
                else:
                    eng.tensor_sub(dst_v, xt, m_t)
                    eng.tensor_mul(dst_v, dst_v, r_t)

        # ---- phase 1: LN1 over the permuted full seq, V interleaved ----
        with nc.named_scope("ln1v"), \
             tc.tile_pool(name="pst1", bufs=2, space="PSUM") as pst, \
             tc.tile_pool(name="pvmm", bufs=2, space="PSUM") as pvm, \
             tc.tile_pool(name="pbc1", bufs=4) as pbc, \
             tc.tile_pool(name="pxh1", bufs=2) as pxh, \
             tc.tile_pool(name="pwv", bufs=1) as pwv:
            wv_sb = [pwv.tile([P, ET, 512], FP8, tag=f"wv{vh}", name=f"wv{vh}")
                     for vh in range(2)]
            for vh in range(2):
                nc.sync.dma_start(
                    out=wv_sb[vh],
                    in_=d["wv"][:, vh * 512:(vh + 1) * 512]
                    .rearrange("(t p) c -> p t c", p=P))
            for blk in range(4):
                c0 = blk * 512
                ln_block(pst, pbc, xnf_h[blk // 2], (c0 % 1024), 512,
                         src_dram=d["x_full_bf"][:, c0:c0 + 512], xh_pool=pxh)
                # V for the 4 key tiles of this block
                for t in range(4 * blk, 4 * blk + 4):
                    xn_src = xnf_h[t // 8]
                    for vh in range(2):
                        hbase = vh * (NH // 2)
                        ps = pvm.tile([P, 512], F32, tag="vmm", name="vmm")
                        for f in range(4):
                            nc.tensor.matmul(
                                ps,
                                xn_src[:, 2 * f:2 * f + 2,
                                       (t % 8) * P:(t % 8 + 1) * P],
                                wv_sb[vh][:, 2 * f:2 * f + 2, :],
                                start=(f == 0), stop=(f == 3),
                                perf_mode=DR)
                        va_v = VA[:, t, hbase * (HD + 1):(hbase + 8) * (HD + 1)] \
                            .rearrange("p (h c) -> p h c", c=HD + 1)
                        with nc.allow_low_precision(reason="fp8 V feed"):
                            nc.vector.tensor_add(
                                va_v[:, :, 0:HD],
                                ps.rearrange("p (h c) -> p h c", c=HD),
                                bvb[:, vh * 512:(vh + 1) * 512]
                                .rearrange("p (h c) -> p h c", c=HD))

        # ---- phase 2: attention; K/Q projections for kd+1 are interleaved
        # one-matmul-per-seg into kd's attention stream so the PE never has
        # a duty-cycle dip (the HAM clock gate halves the PE clock whenever
        # PE activity drops for a while) ----
        with nc.named_scope("attn"), \
             tc.tile_pool(name="pwc", bufs=5) as pw, \
             tc.tile_pool(name="pprobs", bufs=6) as pprobs, \
             tc.tile_pool(name="prb", bufs=4) as prb, \
             tc.tile_pool(name="psc", bufs=4, space="PSUM") as psc, \
             tc.tile_pool(name="pso", bufs=2, space="PSUM") as pso, \
             tc.tile_pool(name="pkq", bufs=2, space="PSUM") as pkq:

            def kq_dma(kdn):
                wts = {}
                for wname in ("wk", "wq"):
                    wt = pw.tile([P, ET, P], FP8, tag="wcol", name="wcol")
                    nc.sync.dma_start(
                        out=wt,
                        in_=d[wname][:, kdn * P:(kdn + 1) * P]
                        .rearrange("(t p) c -> p t c", p=P))
                    wts[wname] = wt
                return wts

            def kq_thunks(kdn, wts):
                th = []
                for (wname, bcol, dst, scols) in (
                        ("wk", bk_sb, KT, S), ("wq", bq_sb, QT, QC)):
                    for c0 in range(0, scols, 512):
                        state = {}
                        def mm(f, wt=wts[wname], c0=c0, state=state):
                            if f == 0:
                                state["ps"] = pkq.tile([P, 512], F32,
                                                       tag="kq", name="kq")
                            src = xnf_h[c0 // 1024][:, 2 * f:2 * f + 2,
                                                    c0 % 1024:c0 % 1024 + 512]
                            nc.tensor.matmul(state["ps"],
                                             wt[:, 2 * f:2 * f + 2, :], src,
                                             start=(f == 0),
                                             stop=(f == 3),
                                             perf_mode=DR,
                                             skip_group_check=True)
                        for f in range(4):
                            th.append(lambda f=f, mm=mm: mm(f))
                        def epi(dst=dst, bcol=bcol, c0=c0, state=state,
                                kdn=kdn):
                            nc.vector.tensor_scalar(
                                dst[:, kdn, c0:c0 + 512], state["ps"],
                                bcol[:, kdn:kdn + 1], None, op0=O.add)
                        th.append(epi)
                return th

            wts = kq_dma(0)
            for th in kq_thunks(0, wts):
                th()

            for kd in range(ET):
                if kd + 1 < ET:
                    wts = kq_dma(kd + 1)
                    thunks = kq_thunks(kd + 1, wts)
                else:
                    thunks = []
                ti = 0
                h0, h1 = 2 * kd, 2 * kd + 1

                def drain(n):
                    nonlocal ti
                    for _ in range(n):
                        if ti < len(thunks):
                            thunks[ti]()
                            ti += 1

                for hf in range(2):
                    npair = 4 if hf == 0 else 8
                    psO = [pso.tile([HD + 1, 512], F32, tag="psO",
                                    name=f"psO{hh}") for hh in range(2)]

                    def emit_pv(j, cs, prs):
                        for hh, h in ((0, h0), (1, h1)):
                            nc.tensor.matmul(
                                psO[hh][:, cs - hf * 512:512],
                                VA[:, j:j + 9:8,
                                   h * (HD + 1):(h + 1) * (HD + 1)],
                                prs[hh],
                                start=(j == 0), stop=(j == npair - 1),
                                perf_mode=DR,
                                skip_group_check=True)

                    pend = None
                    for j in range(npair):
                        cs = max(j * P, hf * 512)
                        ln = (hf + 1) * 512 - cs
                        boundary = (cs == j * P)
                        prs = []
                        for hh, off in ((0, 0), (1, HD)):
                            pr = pprobs.tile([P, 2, 512], FP8, tag="probs",
                                             name="probs")[:, :, :ln]
                            for ji, t in ((0, j), (1, j + 8)):
                                sc = psc.tile([P, 512], F32, tag="sc",
                                              name="sc")[:, :ln]
                                nc.tensor.matmul(
                                    sc,
                                    KT[off:off + HD, kd, t * P:(t + 1) * P],
                                    QT[off:off + HD, kd, cs:cs + ln],
                                    start=True, stop=True)
                                nc.scalar.activation(pr[:, ji, :], sc, A.Exp,
                                                     scale=1.0 / (WS * WS * 8))
                                if boundary:
                                    eng = nc.vector if hh == 0 else nc.gpsimd
                                    msk = tri_sb if ji == 0 else par_sb
                                    with nc.allow_low_precision(
                                            reason="fp8 probs mask"):
                                        eng.tensor_mul(pr[:, ji, 0:P],
                                                       pr[:, ji, 0:P], msk)
                            prs.append(pr)
                        if pend is not None:
                            emit_pv(*pend)
                        pend = (j, cs, prs)
                        drain(3)
                    emit_pv(*pend)
                    # normalization (psO ring keeps the PE fed meanwhile)
                    for hh in range(2):
                        off = hh * HD
                        rrow = prows.tile([1, 512], BF16, tag="rows_bf",
                                          name="rrow")
                        with nc.allow_low_precision(
                                reason="recip row feeds bf16 mul; same "
                                       "precision as f32-recip-then-cast"):
                            nc.vector.reciprocal(rrow, psO[hh][HD:HD + 1, :])
                        rb = prb.tile([HD, 512], BF16, tag="rb", name="rb")
                        nc.gpsimd.partition_broadcast(rb, rrow)
                        with nc.allow_low_precision(reason="fp8 attnT"):
                            nc.vector.tensor_mul(
                                attnT[off:off + HD, kd,
                                      hf * 512:(hf + 1) * 512],
                                psO[hh][0:HD, :], rb)
                drain(len(thunks))

        # ---- phase 3: proj + residual -> x2, LN2 -> xn2 (per q-half) ----
        x2_h = [pbig.tile([P, ET, 512], F32, tag=f"T1{i}", name=f"x2{i}")
                for i in range(2)]
        xn2 = pbig.tile([P, ET, QC], BF16, tag="T4")
        with nc.named_scope("proj"), \
             tc.tile_pool(name="pwp", bufs=3) as pw, \
             tc.tile_pool(name="pppr", bufs=2, space="PSUM") as ppp, \
             tc.tile_pool(name="pst2", bufs=2, space="PSUM") as pst, \
             tc.tile_pool(name="pbc2", bufs=4) as pbc:
            for qh in range(2):
                c0 = qh * 512
                for et in range(ET):
                    wt = pw.tile([P, ET, P], FP8, tag="wcol", name="wcol")
                    nc.sync.dma_start(
                        out=wt,
                        in_=d["wproj"][:, et * P:(et + 1) * P]
                        .rearrange("(t p) c -> p t c", p=P))
                    ps = ppp.tile([P, 512], F32, tag="mm", name="mm")
                    for f in range(4):
                        nc.tensor.matmul(ps, wt[:, 2 * f:2 * f + 2, :],
                                         attnT[:, 2 * f:2 * f + 2,
                                               c0:c0 + 512],
                                         start=(f == 0), stop=(f == 3),
                                         perf_mode=DR)
                    xc = postg.tile([P, 512], F32, tag="ostg", name="ostg")
                    nc.sync.dma_start(
                        out=xc,
                        in_=d["x_chunk"][et * P:(et + 1) * P, c0:c0 + 512])
                    tg = postg.tile([P, 512], F32, tag="ostg", name="ostg")
                    nc.scalar.activation(tg, ps, A.Identity,
                                         bias=bproj_sb[:, et:et + 1],
                                         scale=1.0 / WS)
                    nc.vector.tensor_add(x2_h[qh][:, et, :], tg, xc)
                ln_block(pst, pbc, xn2, c0, 512,
                         src_tiles=lambda et: x2_h[qh][:, et, :])

        # ---- phase 4: FFN, fp8 DoubleRow (2 k-tiles per matmul; weights
        # pre-scaled x16 on the host, folded out via activation scale) ----
        Hsb = [pbig.tile([P, HT, 512], FP8, tag="T3", name="HsbA"),
               pbig.tile([P, HT, 512], FP8, tag="T5", name="HsbB")]
        with nc.named_scope("ffn"), \
             tc.tile_pool(name="pwf", bufs=2) as pwf, \
             tc.tile_pool(name="pwf2", bufs=4) as pwf2, \
             tc.tile_pool(name="ppf1", bufs=2, space="PSUM") as ppf1, \
             tc.tile_pool(name="ppf2", bufs=4, space="PSUM") as ppf2:
            for hg in range(8):
                wt = pwf.tile([P, ET, 512], BF16, tag="wfc1", name="wfc1")
                nc.sync.dma_start(
                    out=wt,
                    in_=d["wfc"][:, hg * 512:(hg + 1) * 512]
                    .rearrange("(t p) c -> p t c", p=P))
                for qh in range(2):
                    for h4 in range(4):
                        ht = hg * 4 + h4
                        ps = ppf1.tile([P, 512], F32, tag="mmh", name="mmh")
                        for et in range(ET):
                            nc.tensor.matmul(
                                ps, wt[:, et, h4 * P:(h4 + 1) * P],
                                xn2[:, et, qh * 512:qh * 512 + 512],
                                start=(et == 0), stop=(et == ET - 1))
                        with nc.allow_low_precision(reason="fp8 Hsb"):
                            nc.scalar.activation(Hsb[qh][:, ht, :], ps,
                                                 A.Gelu,
                                                 bias=bfc_sb[:, ht:ht + 1])
            for qh in range(2):
                for eg in range(2):
                    psY = [ppf2.tile([P, 512], F32, tag="psY",
                                     name=f"psY{i}") for i in range(4)]
                    for htp in range(HT // 2):
                        wt = pwf2.tile([P, 2, 512], FP8, tag="wfc2",
                                       name="wfc2")
                        nc.sync.dma_start(
                            out=wt,
                            in_=d["wfc2"][htp * 256:(htp + 1) * 256,
                                          eg * 512:(eg + 1) * 512]
                            .rearrange("(j p) c -> p j c", p=P))
                        for e4 in range(4):
                            nc.tensor.matmul(
                                psY[e4], wt[:, :, e4 * P:(e4 + 1) * P],
                                Hsb[qh][:, 2 * htp:2 * htp + 2, :],
                                start=(htp == 0), stop=(htp == HT // 2 - 1),
                                perf_mode=DR)
                    for e4 in range(4):
                        et = eg * 4 + e4
                        tg = postg.tile([P, 512], F32, tag="ostg", name="ostg")
                        nc.scalar.activation(tg, psY[e4], A.Identity,
                                             bias=bfc2_sb[:, et:et + 1],
                                             scale=1.0 / WS)
                        og = postg.tile([P, 512], F32, tag="ostg", name="ostg")
                        nc.vector.tensor_add(og, tg, x2_h[qh][:, et, :])
                        nc.sync.dma_start(
                            out=out_ap[et * P:(et + 1) * P,
                                       qh * 512:qh * 512 + 512],
                            in_=og)

# ---------------------------------------------------------------------------
# host side
# ---------------------------------------------------------------------------

_PROG_CACHE = {}


def get_program(repeat=1):
    key = repeat
    if key not in _PROG_CACHE:
        _PROG_CACHE[key] = build_program(repeat)
    return _PROG_CACHE[key]


def _own_rows(parity):
    return np.concatenate(
        [np.arange(P * (2 * j + parity), P * (2 * j + parity) + P)
         for j in range(8)])


def prep_in_maps(x, ln1_g, ln1_b, w_attn, b_attn, w_proj, b_proj,
                 ln2_g, ln2_b, w_fc, b_fc, w_fc2, b_fc2):
    f32 = np.float32
    bf = ml_dtypes.bfloat16
    x = np.asarray(x, f32)
    g1 = np.asarray(ln1_g, f32)[:, None]
    # q/k/v weights+biases carry a x16 fp8 pre-scale; q additionally needs
    # /sqrt(d)=8: both fold into the Exp activation scale 1/(16*16*8) and the
    # softmax-denominator ones column (=16), so everything stays exact.
    wq = g1 * np.asarray(w_attn[:, 0:E], f32)
    wk = g1 * np.asarray(w_attn[:, E:2 * E], f32)
    wv = g1 * np.asarray(w_attn[:, 2 * E:3 * E], f32)
    bq = (np.asarray(w_attn[:, 0:E], f32).T @ np.asarray(ln1_b, f32)
          + np.asarray(b_attn[0:E], f32))
    bk = (np.asarray(w_attn[:, E:2 * E], f32).T @ np.asarray(ln1_b, f32)
          + np.asarray(b_attn[E:2 * E], f32))
    bv = (np.asarray(w_attn[:, 2 * E:3 * E], f32).T @ np.asarray(ln1_b, f32)
          + np.asarray(b_attn[2 * E:3 * E], f32))
    g2 = np.asarray(ln2_g, f32)[:, None]
    wfc = g2 * np.asarray(w_fc, f32)
    bfc = np.asarray(w_fc, f32).T @ np.asarray(ln2_b, f32) + np.asarray(b_fc, f32)

    fp8 = ml_dtypes.float8_e4m3
    def to_fp8(w):
        return np.ascontiguousarray(
            np.clip(w * WS, -240.0, 240.0).astype(fp8))

    shared = {
        "wq": to_fp8(wq),
        "wk": to_fp8(wk),
        "wv": to_fp8(wv),
        "wproj": to_fp8(np.asarray(w_proj, f32)),
        "wfc": np.ascontiguousarray(wfc.astype(bf)),
        "wfc2": to_fp8(np.asarray(w_fc2, f32)),
        "bq": np.ascontiguousarray((bq * WS).astype(f32)),
        "bk": np.ascontiguousarray((bk * WS).astype(f32)),
        "bv": np.ascontiguousarray((bv * WS).astype(f32)),
        "bproj": np.ascontiguousarray(np.asarray(b_proj, f32)),
        "bfc": np.ascontiguousarray(bfc.astype(f32)),
        "bfc2": np.ascontiguousarray(np.asarray(b_fc2, f32)),
    }

    tri = (np.arange(P)[:, None] <= np.arange(P)[None, :]).astype(np.float32)
    tri = np.ascontiguousarray(tri.astype(bf))

    in_maps = []
    for c in range(N_CORES):
        b, parity = c % BATCH, c // BATCH
        rows_own = _own_rows(parity)
        rows_par = _own_rows(1 - parity)
        perm = np.concatenate([rows_own, rows_par])
        xbt = np.ascontiguousarray(x[b].T)          # [E, S]
        m = dict(shared)
        m["x_full_bf"] = np.ascontiguousarray(xbt[:, perm].astype(bf))
        m["x_chunk"] = np.ascontiguousarray(xbt[:, rows_own])
        m["tri_mask"] = tri
        m["par_mask"] = np.ascontiguousarray(
            np.full((P, P), float(parity), np.float32).astype(bf))
        in_maps.append(m)
    return in_maps


def assemble_output(results):
    y = np.empty((BATCH, S, E), np.float32)
    for c in range(N_CORES):
        b, parity = c % BATCH, c // BATCH
        y[b, _own_rows(parity), :] = results[c]["out"].T
    return y


def kernel(**inputs):
    nc = get_program(1)
    in_maps = prep_in_maps(**inputs)
    res = run_bass_kernel_spmd(nc, in_maps, core_ids=list(range(N_CORES)))
    return assemble_output(res.results)
